# revision 41
# baseline (speedup 1.0000x reference)
"""Multi-head attention forward (B=16, S=1024, d=1024, H=16, Dh=64) on 8
Trainium2 NeuronCores, data-parallel over batch (2 batches per core).

Device kernel (per core, bf16 matmuls, fp32 accumulate):
  inputs (host-prepped): XT [d, 2048] = hidden[2c:2c+2].reshape(2048,d).T,
  WqT/WkT/WvT = W.T [in, out], WoT = Wo.T [dv, o]  (all bf16),
  bq, bk [1024] f32, bo2 = bo + Wo @ bv  (bv folded: softmax rows sum to 1).

Key structure (vs earlier version): query chunks (c) are OUTER, head pairs
(j) inner.  Scores for a head pair go into ONE [128,1024] PSUM tile
(h0 -> bank A cols 0:512, h1 -> bank B) so the two 64x128 row-tiled matmuls
become ready together and execute CONCURRENTLY on the PE (row tiles 0/64),
and a single [128,1024] exp drains both.  pvps needs only 2 banks per (c,j)
so PSUM = scores ring 4 + pv 2 + proj 2 = 8 banks.  Projections / V-proj /
out-proj are emitted as 8-matmul fill units drained between score blocks.
"""

import numpy as np
import ml_dtypes

import concourse.bass as bass
import concourse.mybir as mybir
import concourse.tile as tile
from concourse import bacc
from concourse.bass_utils import run_bass_kernel_spmd

P = 128
D = 1024
T = 2048  # tokens per core
TB = 1024  # tokens per batch (= S)
H = 16
DH = 64
KD = D // P  # 8 partition-tiles of the d/dv/s dims
NB = T // TB  # batches per core
CW = 512  # query-chunk width (one PSUM bank of fp32)
NCH = TB // CW  # 2 query chunks per batch
NCORES = 8

BF16 = mybir.dt.bfloat16
F32 = mybir.dt.float32
EXPF = mybir.ActivationFunctionType.Exp
MULT = mybir.AluOpType.mult

# test.py hooks
TRACE = False
TRACE_KWARGS = {}
LAST_RESULTS = None

_NC_CACHE = None


def build_nc():
    from collections import deque
    from contextlib import ExitStack

    nc = bacc.Bacc("TRN2", target_bir_lowering=False, debug=False, num_devices=NCORES)

    xt_d = nc.dram_tensor("xt", [D, T], BF16, kind="ExternalInput")
    wqt_d = nc.dram_tensor("wqt", [D, D], BF16, kind="ExternalInput")
    wkt_d = nc.dram_tensor("wkt", [D, D], BF16, kind="ExternalInput")
    wvt_d = nc.dram_tensor("wvt", [D, D], BF16, kind="ExternalInput")
    wot_d = nc.dram_tensor("wot", [D, D], BF16, kind="ExternalInput")
    bq_d = nc.dram_tensor("bq", [D], F32, kind="ExternalInput")
    bk_d = nc.dram_tensor("bk", [D], F32, kind="ExternalInput")
    bo2_d = nc.dram_tensor("bo2", [D], F32, kind="ExternalInput")
    outt_d = nc.dram_tensor("outt", [D, T], F32, kind="ExternalOutput")

    with tile.TileContext(nc) as tc:
        with ExitStack() as ctx:
            # PSUM first so the 2-bank scores tiles land bank-aligned.
            scp = ctx.enter_context(tc.tile_pool(name="sc", bufs=2, space="PSUM"))
            pvp = ctx.enter_context(tc.tile_pool(name="pv", bufs=1, space="PSUM"))
            prp = ctx.enter_context(tc.tile_pool(name="pr", bufs=2, space="PSUM"))
            wpool = ctx.enter_context(tc.tile_pool(name="w", bufs=1))
            xpool = ctx.enter_context(tc.tile_pool(name="x", bufs=1))
            qkpool = ctx.enter_context(tc.tile_pool(name="qk", bufs=1))
            vpool = ctx.enter_context(tc.tile_pool(name="v", bufs=2))
            ptpool = ctx.enter_context(tc.tile_pool(name="pt", bufs=4))
            cpool = ctx.enter_context(tc.tile_pool(name="ctx", bufs=2))
            npool = ctx.enter_context(tc.tile_pool(name="norm", bufs=2))
            opool = ctx.enter_context(tc.tile_pool(name="out", bufs=2))
            spool = ctx.enter_context(tc.tile_pool(name="small", bufs=1))

            # ---- global tiles + DMA loads (interleaved for fast rampup) ----
            xt = [xpool.tile([P, T], BF16, tag=f"xt{k}", name=f"xt{k}") for k in range(KD)]
            wq, wk, wv, wo = (
                [wpool.tile([P, D], BF16, tag=f"w{nm}{k}", name=f"w{nm}{k}") for k in range(KD)]
                for nm in "qkvo"
            )
            # batch-0 halves of xt + whole wq/wk first (unblocks first QK
            # proj), then wv (V-proj), then batch-1 xt halves, wo last.
            for k in range(KD):
                nc.sync.dma_start(xt[k][:, 0:TB], xt_d[k * P : (k + 1) * P, 0:TB])
                nc.sync.dma_start(wq[k][:], wqt_d[k * P : (k + 1) * P, :])
                nc.sync.dma_start(wk[k][:], wkt_d[k * P : (k + 1) * P, :])
            bq_sb = spool.tile([P, KD], F32, tag="bq", name="bq_sb")
            bk_sb = spool.tile([P, KD], F32, tag="bk", name="bk_sb")
            bo_sb = spool.tile([P, KD], F32, tag="bo", name="bo_sb")
            for sb, dr in ((bq_sb, bq_d), (bk_sb, bk_d)):
                nc.sync.dma_start(sb[:], dr.rearrange("(o p) -> p o", p=P))
            for k in range(KD):
                nc.sync.dma_start(wv[k][:], wvt_d[k * P : (k + 1) * P, :])
            nc.sync.dma_start(bo_sb[:], bo2_d.rearrange("(o p) -> p o", p=P))
            for k in range(KD):
                nc.sync.dma_start(xt[k][:, TB:T], xt_d[k * P : (k + 1) * P, TB:T])
            for k in range(KD):
                nc.sync.dma_start(wo[k][:], wot_d[k * P : (k + 1) * P, :])
            # h1-half of wo's last tile, staged at partitions 0:64 so the tail
            # can consume the final head's ctx without the partition-shift DMA
            wo7b = wpool.tile([DH, D], BF16, tag="wo7b", name="wo7b")
            nc.sync.dma_start(wo7b[:], wot_d[D - DH : D, :])

            # ---- persistent per-batch state ----
            vt = {}  # (b, mt) -> v tile [P, H, DH+1]
            kt = {}  # (b, j) -> K^T tile [P, TB]
            qt = {}  # (b, j, c) -> Q^T chunk tile [P, CW]
            ctxts = {}  # (b, j) -> ctx^T tile [P, TB]

            fill = deque()  # (key, closure)
            done = set()

            def push(key, closure):
                fill.append((key, closure))

            def drain(n):
                for _ in range(min(n, len(fill))):
                    k, f = fill.popleft()
                    f()
                    done.add(k)

            def need(key):
                # force-drain (in FIFO order) until `key` has been emitted;
                # guarantees emission-order dependencies for dict tiles.
                while key not in done:
                    assert fill, f"need({key}) but fill queue empty"
                    k, f = fill.popleft()
                    f()
                    done.add(k)

            # ---- fill units (each ~8 matmuls + epilogue) ----
            def v_unit(b, mt, ch):
                def emit():
                    if (b, mt) not in vt:
                        vt[(b, mt)] = vpool.tile(
                            [P, H, DH + 1], BF16, tag=f"v{mt}", name=f"v{mt}", bufs=2
                        )
                        nc.vector.memset(vt[(b, mt)][:, :, DH : DH + 1], 1.0)
                    ps = prp.tile([P, CW], F32, tag="pr", name="prv")
                    for k in range(KD):
                        nc.tensor.matmul(
                            ps[:],
                            xt[k][:, (b * KD + mt) * P : (b * KD + mt + 1) * P],
                            wv[k][:, ch * CW : (ch + 1) * CW],
                            start=(k == 0),
                            stop=(k == KD - 1),
                        )
                    nc.vector.tensor_copy(
                        vt[(b, mt)][:, ch * 8 : (ch + 1) * 8, 0:DH],
                        ps.rearrange("p (h d) -> p h d", d=DH),
                    )

                return emit

            def k_unit(b, j, ch):
                def emit():
                    if (b, j) not in kt:
                        kt[(b, j)] = qkpool.tile(
                            [P, TB], BF16, tag=f"k{j}", name=f"kt{j}", bufs=1
                        )
                    ps = prp.tile([P, CW], F32, tag="pr", name="prk")
                    for k in range(KD):
                        nc.tensor.matmul(
                            ps[:],
                            wk[k][:, j * P : (j + 1) * P],
                            xt[k][:, b * TB + ch * CW : b * TB + (ch + 1) * CW],
                            start=(k == 0),
                            stop=(k == KD - 1),
                        )
                    nc.vector.tensor_scalar_add(
                        kt[(b, j)][:, ch * CW : (ch + 1) * CW], ps[:], bk_sb[:, j : j + 1]
                    )

                return emit

            def q_unit(b, j, c):
                def emit():
                    qt[(b, j, c)] = qkpool.tile(
                        [P, CW], BF16, tag=f"q{j}", name=f"qt{j}", bufs=1
                    )
                    ps = prp.tile([P, CW], F32, tag="pr", name="prq")
                    for k in range(KD):
                        nc.tensor.matmul(
                            ps[:],
                            wq[k][:, j * P : (j + 1) * P],
                            xt[k][:, b * TB + c * CW : b * TB + (c + 1) * CW],
                            start=(k == 0),
                            stop=(k == KD - 1),
                        )
                    nc.vector.tensor_scalar_add(
                        qt[(b, j, c)][:], ps[:], bq_sb[:, j : j + 1]
                    )

                return emit

            def out_unit(b, c, mo, split_last=False):
                def emit():
                    ps = prp.tile([P, CW], F32, tag="pr", name="pro")
                    last = KD - 1
                    for k in range(last):
                        nc.tensor.matmul(
                            ps[:],
                            wo[k][:, mo * P : (mo + 1) * P],
                            ctxts[(b, k)][:, c * CW : (c + 1) * CW],
                            start=(k == 0),
                            stop=False,
                        )
                    if split_last:
                        # final tile in two 64-deep halves, both at row-tile
                        # position 0 (same row group -> serialized, no PSUM
                        # bank race); h1 comes from the direct staging tile
                        nc.tensor.matmul(
                            ps[:],
                            wo[last][0:DH, mo * P : (mo + 1) * P],
                            ctxts[(b, last)][0:DH, c * CW : (c + 1) * CW],
                            start=False,
                            stop=False,
                        )
                        nc.tensor.matmul(
                            ps[:],
                            wo7b[:, mo * P : (mo + 1) * P],
                            ctx_h1_last[0],
                            start=False,
                            stop=True,
                        )
                    else:
                        nc.tensor.matmul(
                            ps[:],
                            wo[last][:, mo * P : (mo + 1) * P],
                            ctxts[(b, last)][:, c * CW : (c + 1) * CW],
                            start=False,
                            stop=True,
                        )
                    osb = opool.tile([P, CW], F32, tag="osb", name="osb")
                    nc.vector.tensor_scalar_add(osb[:], ps[:], bo_sb[:, mo : mo + 1])
                    nc.sync.dma_start(
                        outt_d[
                            mo * P : (mo + 1) * P,
                            b * TB + c * CW : b * TB + (c + 1) * CW,
                        ],
                        osb[:],
                    )

                return emit

            # ---- attention inner loop ----
            ctx_h1_last = [None]

            def normalize(b, c, j, pva, pvb):
                if (b, j) not in ctxts:
                    ctxts[(b, j)] = cpool.tile(
                        [P, TB], BF16, tag=f"ctxt{j}", name=f"ctxt{j}", bufs=2
                    )
                ctile = ctxts[(b, j)]
                final = b == NB - 1 and c == NCH - 1 and j == KD - 1
                for h, pv_t in ((0, pva), (1, pvb)):
                    rs = npool.tile([1, CW], F32, tag="rs", name="rs", bufs=1)
                    nc.vector.tensor_copy(rs[:], pv_t[DH : DH + 1, :])
                    rr = npool.tile([1, CW], F32, tag="rr", name="rr", bufs=1)
                    nc.vector.reciprocal_approx_fast(rr[:], rs[:])
                    rb = npool.tile([DH, CW], F32, tag="rb", name="rb", bufs=1)
                    nc.gpsimd.partition_broadcast(rb[:], rr[:])
                    if h == 0:
                        nc.vector.tensor_tensor(
                            ctile[0:DH, c * CW : (c + 1) * CW],
                            pv_t[0:DH, :],
                            rb[:],
                            MULT,
                        )
                    elif final:
                        # keep the very last head's ctx at partitions 0:64 so
                        # the tail consumes it without the shift DMA
                        chl = npool.tile([DH, CW], BF16, tag="chl", name="chl", bufs=1)
                        nc.vector.tensor_tensor(chl[:], pv_t[0:DH, :], rb[:], MULT)
                        ctx_h1_last[0] = chl
                    else:
                        ch = npool.tile([DH, CW], BF16, tag="ch", name="ch", bufs=1)
                        nc.vector.tensor_tensor(ch[:], pv_t[0:DH, :], rb[:], MULT)
                        nc.sync.dma_start(
                            ctile[DH:P, c * CW : (c + 1) * CW], ch[:]
                        )

            def attention_cj(b, c, j):
                need(("q", b, j, c))
                need(("k", b, j, 0))
                need(("k", b, j, 1))
                pva = pvp.tile([P, CW], F32, tag="pva", name="pva", bufs=1)
                pvb = pvp.tile([P, CW], F32, tag="pvb", name="pvb", bufs=1)
                ktj = kt[(b, j)]
                qjc = qt[(b, j, c)]
                pts = []
                for blk in range(KD // 2):
                    two = (2 * blk, 2 * blk + 1)
                    for st in two:
                        # one [128,1024] 2-bank tile per head-pair: both 64x128
                        # row-tiled matmuls become ready together -> execute
                        # concurrently; one exp drains both banks.
                        sc = scp.tile([P, 2 * CW], F32, tag="sc", name="sc", bufs=2)
                        for h in range(2):
                            r0 = h * DH
                            nc.tensor.matmul(
                                sc[:, h * CW : (h + 1) * CW],
                                ktj[r0 : r0 + DH, st * P : (st + 1) * P],
                                qjc[r0 : r0 + DH, :],
                                start=True,
                                stop=True,
                            )
                        pt = ptpool.tile([P, 2 * CW], BF16, tag="pt", name="pt", bufs=4)
                        nc.scalar.activation(pt[:], sc[:], EXPF, scale=0.125)
                        pts.append(pt)
                    for st in two:
                        need(("v", b, st, j // 4))
                        for h, pv_t in ((0, pva), (1, pvb)):
                            nc.tensor.matmul(
                                pv_t[0 : DH + 1, :],
                                vt[(b, st)][:, 2 * j + h, :],
                                pts[st][:, h * CW : (h + 1) * CW],
                                start=(st == 0),
                                stop=(st == KD - 1),
                            )
                    drain(1)
                normalize(b, c, j, pva, pvb)

            # ---- head: QK proj of (b0, j0) emitted directly; V + j1 proj
            # queued so early score pairs preempt them by priority ----
            for key, u in (
                (("q", 0, 0, 0), q_unit(0, 0, 0)),
                (("k", 0, 0, 0), k_unit(0, 0, 0)),
                (("k", 0, 0, 1), k_unit(0, 0, 1)),
            ):
                u()
                done.add(key)
            push(("q", 0, 1, 0), q_unit(0, 1, 0))
            push(("k", 0, 1, 0), k_unit(0, 1, 0))
            push(("k", 0, 1, 1), k_unit(0, 1, 1))
            for mt in range(KD):
                push(("v", 0, mt, 0), v_unit(0, mt, 0))
            for mt in range(KD):
                push(("v", 0, mt, 1), v_unit(0, mt, 1))

            # ---- main loops ----
            for b in range(NB):
                for c in range(NCH):
                    for j in range(KD):
                        # schedule fill production
                        if c == 0:
                            if j < KD - 1:
                                if not (b == 0 and j == 0):  # j1 pre-queued in head
                                    push(("q", b, j + 1, 0), q_unit(b, j + 1, 0))
                                    push(("k", b, j + 1, 0), k_unit(b, j + 1, 0))
                                    push(("k", b, j + 1, 1), k_unit(b, j + 1, 1))
                            else:
                                push(("q", b, 0, 1), q_unit(b, 0, 1))
                            if j == 6 and b + 1 < NB:
                                # next batch V-proj early: feeds the thin
                                # c0-phase tail blocks
                                for mt in range(KD):
                                    push(("v", b + 1, mt, 0), v_unit(b + 1, mt, 0))
                        else:
                            if j < KD - 1:
                                push(("q", b, j + 1, 1), q_unit(b, j + 1, 1))
                            if j == 0 and b + 1 < NB:
                                for mt in range(KD):
                                    push(("v", b + 1, mt, 1), v_unit(b + 1, mt, 1))
                                    if mt % 2 == 0:
                                        push(("o", b, 0, mt // 2), out_unit(b, 0, mt // 2))
                                    else:
                                        push(("o", b, 0, mt // 2 + 4), out_unit(b, 0, mt // 2 + 4))
                            if j == 0 and b + 1 == NB:
                                for mo in range(KD):
                                    push(("o", b, 0, mo), out_unit(b, 0, mo))
                            if j == 4 and b + 1 < NB:
                                push(("q", b + 1, 0, 0), q_unit(b + 1, 0, 0))
                                push(("k", b + 1, 0, 0), k_unit(b + 1, 0, 0))
                                push(("k", b + 1, 0, 1), k_unit(b + 1, 0, 1))
                        if b == 1 and c == 0 and j == 0:
                            for mo in range(4):
                                push(("o", 0, 1, mo), out_unit(0, 1, mo))
                        if b == 1 and c == 0 and j == 5:
                            for mo in range(4, KD):
                                push(("o", 0, 1, mo), out_unit(0, 1, mo))
                        attention_cj(b, c, j)

            # ---- tail: last batch / last chunk output projection ----
            drain(len(fill))
            for mo in range(KD):
                out_unit(NB - 1, NCH - 1, mo, split_last=True)()

    nc.compile()
    return nc


def _get_nc():
    global _NC_CACHE
    if _NC_CACHE is None:
        _NC_CACHE = build_nc()
    return _NC_CACHE


def kernel(hidden_states, Wq, bq, Wk, bk, Wv, bv, Wo, bo):
    global LAST_RESULTS
    bf = ml_dtypes.bfloat16
    hs = np.asarray(hidden_states, np.float32)
    Wq = np.asarray(Wq, np.float32)
    Wk = np.asarray(Wk, np.float32)
    Wv = np.asarray(Wv, np.float32)
    Wo = np.asarray(Wo, np.float32)
    bq = np.asarray(bq, np.float32)
    bk = np.asarray(bk, np.float32)
    bv = np.asarray(bv, np.float32)
    bo = np.asarray(bo, np.float32)

    wqt = np.ascontiguousarray(Wq.T).astype(bf)
    wkt = np.ascontiguousarray(Wk.T).astype(bf)
    wvt = np.ascontiguousarray(Wv.T).astype(bf)
    wot = np.ascontiguousarray(Wo.T).astype(bf)
    bo2 = (bo + Wo @ bv).astype(np.float32)

    bpc = hs.shape[0] // NCORES  # batches per core
    in_maps = []
    for c in range(NCORES):
        xc = hs[c * bpc : (c + 1) * bpc].reshape(bpc * TB, D)
        in_maps.append(
            {
                "xt": np.ascontiguousarray(xc.T).astype(bf),
                "wqt": wqt,
                "wkt": wkt,
                "wvt": wvt,
                "wot": wot,
                "bq": bq,
                "bk": bk,
                "bo2": bo2,
            }
        )

    nc = _get_nc()
    res = run_bass_kernel_spmd(
        nc,
        in_maps,
        core_ids=list(range(NCORES)),
        trace=TRACE,
        **TRACE_KWARGS,
    )
    LAST_RESULTS = res

    out = np.empty((hs.shape[0], TB, D), np.float32)
    for c in range(NCORES):
        ot = res.results[c]["outt"]  # [D, T]
        for b in range(bpc):
            out[c * bpc + b] = ot[:, b * TB : (b + 1) * TB].T
    return out


# revision 42
# speedup vs baseline: 1.0349x; 1.0349x over previous
"""Multi-head attention forward (B=16, S=1024, d=1024, H=16, Dh=64) on 8
Trainium2 NeuronCores, data-parallel over batch (2 batches per core).

Device kernel (per core, bf16 matmuls, fp32 accumulate):
  inputs (host-prepped): XT [d, 2048] = hidden[2c:2c+2].reshape(2048,d).T,
  WqT/WkT/WvT = W.T [in, out], WoT = Wo.T [dv, o]  (all bf16),
  bq, bk [1024] f32, bo2 = bo + Wo @ bv  (bv folded: softmax rows sum to 1).

Key structure (vs earlier version): query chunks (c) are OUTER, head pairs
(j) inner.  Scores for a head pair go into ONE [128,1024] PSUM tile
(h0 -> bank A cols 0:512, h1 -> bank B) so the two 64x128 row-tiled matmuls
become ready together and execute CONCURRENTLY on the PE (row tiles 0/64),
and a single [128,1024] exp drains both.  pvps needs only 2 banks per (c,j)
so PSUM = scores ring 4 + pv 2 + proj 2 = 8 banks.  Projections / V-proj /
out-proj are emitted as 8-matmul fill units drained between score blocks.
"""

import numpy as np
import ml_dtypes

import concourse.bass as bass
import concourse.mybir as mybir
import concourse.tile as tile
from concourse import bacc
from concourse.bass_utils import run_bass_kernel_spmd

P = 128
D = 1024
T = 2048  # tokens per core
TB = 1024  # tokens per batch (= S)
H = 16
DH = 64
KD = D // P  # 8 partition-tiles of the d/dv/s dims
NB = T // TB  # batches per core
CW = 512  # query-chunk width (one PSUM bank of fp32)
NCH = TB // CW  # 2 query chunks per batch
NCORES = 8

BF16 = mybir.dt.bfloat16
F32 = mybir.dt.float32
EXPF = mybir.ActivationFunctionType.Exp
MULT = mybir.AluOpType.mult

# test.py hooks
TRACE = False
TRACE_KWARGS = {}
LAST_RESULTS = None

_NC_CACHE = None


def build_nc():
    from collections import deque
    from contextlib import ExitStack

    nc = bacc.Bacc("TRN2", target_bir_lowering=False, debug=False, num_devices=NCORES)

    xt_d = nc.dram_tensor("xt", [D, T], BF16, kind="ExternalInput")
    wqt_d = nc.dram_tensor("wqt", [D, D], BF16, kind="ExternalInput")
    wkt_d = nc.dram_tensor("wkt", [D, D], BF16, kind="ExternalInput")
    wvt_d = nc.dram_tensor("wvt", [D, D], BF16, kind="ExternalInput")
    wot_d = nc.dram_tensor("wot", [D, D], BF16, kind="ExternalInput")
    bq_d = nc.dram_tensor("bq", [D], F32, kind="ExternalInput")
    bk_d = nc.dram_tensor("bk", [D], F32, kind="ExternalInput")
    bo2_d = nc.dram_tensor("bo2", [D], F32, kind="ExternalInput")
    outt_d = nc.dram_tensor("outt", [D, T], F32, kind="ExternalOutput")

    with tile.TileContext(nc) as tc:
        with ExitStack() as ctx:
            # PSUM first so the 2-bank scores tiles land bank-aligned.
            scp = ctx.enter_context(tc.tile_pool(name="sc", bufs=2, space="PSUM"))
            pvp = ctx.enter_context(tc.tile_pool(name="pv", bufs=1, space="PSUM"))
            prp = ctx.enter_context(tc.tile_pool(name="pr", bufs=2, space="PSUM"))
            wpool = ctx.enter_context(tc.tile_pool(name="w", bufs=1))
            xpool = ctx.enter_context(tc.tile_pool(name="x", bufs=1))
            qkpool = ctx.enter_context(tc.tile_pool(name="qk", bufs=1))
            vpool = ctx.enter_context(tc.tile_pool(name="v", bufs=2))
            ptpool = ctx.enter_context(tc.tile_pool(name="pt", bufs=4))
            cpool = ctx.enter_context(tc.tile_pool(name="ctx", bufs=2))
            npool = ctx.enter_context(tc.tile_pool(name="norm", bufs=2))
            opool = ctx.enter_context(tc.tile_pool(name="out", bufs=2))
            spool = ctx.enter_context(tc.tile_pool(name="small", bufs=1))

            # ---- global tiles + DMA loads (interleaved for fast rampup) ----
            xt = [xpool.tile([P, T], BF16, tag=f"xt{k}", name=f"xt{k}") for k in range(KD)]
            wq, wk, wv, wo = (
                [wpool.tile([P, D], BF16, tag=f"w{nm}{k}", name=f"w{nm}{k}") for k in range(KD)]
                for nm in "qkvo"
            )
            # batch-0 halves of xt + whole wq/wk first (unblocks first QK
            # proj), then wv (V-proj), then batch-1 xt halves, wo last.
            for k in range(KD):
                nc.sync.dma_start(xt[k][:, 0:TB], xt_d[k * P : (k + 1) * P, 0:TB])
                nc.sync.dma_start(wq[k][:], wqt_d[k * P : (k + 1) * P, :])
                nc.sync.dma_start(wk[k][:], wkt_d[k * P : (k + 1) * P, :])
            bq_sb = spool.tile([P, KD], F32, tag="bq", name="bq_sb")
            bk_sb = spool.tile([P, KD], F32, tag="bk", name="bk_sb")
            bo_sb = spool.tile([P, KD], F32, tag="bo", name="bo_sb")
            for sb, dr in ((bq_sb, bq_d), (bk_sb, bk_d)):
                nc.sync.dma_start(sb[:], dr.rearrange("(o p) -> p o", p=P))
            for k in range(KD):
                nc.sync.dma_start(wv[k][:], wvt_d[k * P : (k + 1) * P, :])
            nc.sync.dma_start(bo_sb[:], bo2_d.rearrange("(o p) -> p o", p=P))
            for k in range(KD):
                nc.sync.dma_start(xt[k][:, TB:T], xt_d[k * P : (k + 1) * P, TB:T])
            for k in range(KD):
                nc.sync.dma_start(wo[k][:], wot_d[k * P : (k + 1) * P, :])
            # h1-half of wo's last tile, staged at partitions 0:64 so the tail
            # can consume the final head's ctx without the partition-shift DMA
            wo7b = wpool.tile([DH, D], BF16, tag="wo7b", name="wo7b")
            nc.sync.dma_start(wo7b[:], wot_d[D - DH : D, :])

            # ---- persistent per-batch state ----
            vt = {}  # (b, mt) -> v tile [P, H, DH+1]
            kt = {}  # (b, j) -> K^T tile [P, TB]
            qt = {}  # (b, j, c) -> Q^T chunk tile [P, CW]
            ctxts = {}  # (b, j) -> ctx^T tile [P, TB]

            fill = deque()  # (key, closure)
            done = set()

            def push(key, closure):
                fill.append((key, closure))

            def drain(n):
                for _ in range(min(n, len(fill))):
                    k, f = fill.popleft()
                    f()
                    done.add(k)

            def need(key):
                # force-drain (in FIFO order) until `key` has been emitted;
                # guarantees emission-order dependencies for dict tiles.
                while key not in done:
                    assert fill, f"need({key}) but fill queue empty"
                    k, f = fill.popleft()
                    f()
                    done.add(k)

            # ---- fill units (each ~8 matmuls + epilogue) ----
            def v_unit(b, mt, ch):
                def emit():
                    if (b, mt) not in vt:
                        vt[(b, mt)] = vpool.tile(
                            [P, H, DH + 1], BF16, tag=f"v{mt}", name=f"v{mt}", bufs=2
                        )
                        nc.vector.memset(vt[(b, mt)][:, :, DH : DH + 1], 1.0)
                    ps = prp.tile([P, CW], F32, tag="pr", name="prv")
                    for k in range(KD):
                        nc.tensor.matmul(
                            ps[:],
                            xt[k][:, (b * KD + mt) * P : (b * KD + mt + 1) * P],
                            wv[k][:, ch * CW : (ch + 1) * CW],
                            start=(k == 0),
                            stop=(k == KD - 1),
                        )
                    nc.vector.tensor_copy(
                        vt[(b, mt)][:, ch * 8 : (ch + 1) * 8, 0:DH],
                        ps.rearrange("p (h d) -> p h d", d=DH),
                    )

                return emit

            def k_unit(b, j, ch):
                def emit():
                    if (b, j) not in kt:
                        kt[(b, j)] = qkpool.tile(
                            [P, TB], BF16, tag=f"k{j}", name=f"kt{j}", bufs=1
                        )
                    ps = prp.tile([P, CW], F32, tag="pr", name="prk")
                    for k in range(KD):
                        nc.tensor.matmul(
                            ps[:],
                            wk[k][:, j * P : (j + 1) * P],
                            xt[k][:, b * TB + ch * CW : b * TB + (ch + 1) * CW],
                            start=(k == 0),
                            stop=(k == KD - 1),
                        )
                    nc.vector.tensor_scalar_add(
                        kt[(b, j)][:, ch * CW : (ch + 1) * CW], ps[:], bk_sb[:, j : j + 1]
                    )

                return emit

            def q_unit(b, j, c):
                def emit():
                    qt[(b, j, c)] = qkpool.tile(
                        [P, CW], BF16, tag=f"q{j}", name=f"qt{j}", bufs=1
                    )
                    ps = prp.tile([P, CW], F32, tag="pr", name="prq")
                    for k in range(KD):
                        nc.tensor.matmul(
                            ps[:],
                            wq[k][:, j * P : (j + 1) * P],
                            xt[k][:, b * TB + c * CW : b * TB + (c + 1) * CW],
                            start=(k == 0),
                            stop=(k == KD - 1),
                        )
                    nc.vector.tensor_scalar_add(
                        qt[(b, j, c)][:], ps[:], bq_sb[:, j : j + 1]
                    )

                return emit

            def out_unit(b, c, mo, split_last=False):
                def emit():
                    ps = prp.tile([P, CW], F32, tag="pr", name="pro")
                    last = KD - 1
                    for k in range(last):
                        nc.tensor.matmul(
                            ps[:],
                            wo[k][:, mo * P : (mo + 1) * P],
                            ctxts[(b, k)][:, c * CW : (c + 1) * CW],
                            start=(k == 0),
                            stop=False,
                        )
                    if split_last:
                        # final tile in two 64-deep halves, both at row-tile
                        # position 0 (same row group -> serialized, no PSUM
                        # bank race); h1 comes from the direct staging tile
                        nc.tensor.matmul(
                            ps[:],
                            wo[last][0:DH, mo * P : (mo + 1) * P],
                            ctxts[(b, last)][0:DH, c * CW : (c + 1) * CW],
                            start=False,
                            stop=False,
                        )
                        nc.tensor.matmul(
                            ps[:],
                            wo7b[:, mo * P : (mo + 1) * P],
                            ctx_h1_last[0],
                            start=False,
                            stop=True,
                        )
                    else:
                        nc.tensor.matmul(
                            ps[:],
                            wo[last][:, mo * P : (mo + 1) * P],
                            ctxts[(b, last)][:, c * CW : (c + 1) * CW],
                            start=False,
                            stop=True,
                        )
                    osb = opool.tile([P, CW], F32, tag="osb", name="osb")
                    nc.vector.tensor_scalar_add(osb[:], ps[:], bo_sb[:, mo : mo + 1])
                    nc.sync.dma_start(
                        outt_d[
                            mo * P : (mo + 1) * P,
                            b * TB + c * CW : b * TB + (c + 1) * CW,
                        ],
                        osb[:],
                    )

                return emit

            # ---- attention inner loop ----
            ctx_h1_last = [None]

            def normalize(b, c, j, pva, pvb):
                if (b, j) not in ctxts:
                    ctxts[(b, j)] = cpool.tile(
                        [P, TB], BF16, tag=f"ctxt{j}", name=f"ctxt{j}", bufs=2
                    )
                ctile = ctxts[(b, j)]
                final = b == NB - 1 and c == NCH - 1 and j == KD - 1
                for h, pv_t in ((0, pva), (1, pvb)):
                    rs = npool.tile([1, CW], F32, tag="rs", name="rs", bufs=1)
                    nc.vector.tensor_copy(rs[:], pv_t[DH : DH + 1, :])
                    rr = npool.tile([1, CW], F32, tag="rr", name="rr", bufs=1)
                    nc.vector.reciprocal_approx_fast(rr[:], rs[:])
                    rb = npool.tile([DH, CW], F32, tag="rb", name="rb", bufs=1)
                    nc.gpsimd.partition_broadcast(rb[:], rr[:])
                    if h == 0:
                        nc.vector.tensor_tensor(
                            ctile[0:DH, c * CW : (c + 1) * CW],
                            pv_t[0:DH, :],
                            rb[:],
                            MULT,
                        )
                    elif final:
                        # keep the very last head's ctx at partitions 0:64 so
                        # the tail consumes it without the shift DMA
                        chl = npool.tile([DH, CW], BF16, tag="chl", name="chl", bufs=1)
                        nc.vector.tensor_tensor(chl[:], pv_t[0:DH, :], rb[:], MULT)
                        ctx_h1_last[0] = chl
                    else:
                        ch = npool.tile([DH, CW], BF16, tag="ch", name="ch", bufs=1)
                        nc.vector.tensor_tensor(ch[:], pv_t[0:DH, :], rb[:], MULT)
                        nc.sync.dma_start(
                            ctile[DH:P, c * CW : (c + 1) * CW], ch[:]
                        )

            def attention_cj(b, c, j):
                need(("q", b, j, c))
                need(("k", b, j, 0))
                need(("k", b, j, 1))
                pva = pvp.tile([P, CW], F32, tag="pva", name="pva", bufs=1)
                pvb = pvp.tile([P, CW], F32, tag="pvb", name="pvb", bufs=1)
                ktj = kt[(b, j)]
                qjc = qt[(b, j, c)]
                def pv_two(two):
                    for st in two:
                        need(("v", b, st, j // 4))
                        for h, pv_t in ((0, pva), (1, pvb)):
                            nc.tensor.matmul(
                                pv_t[0 : DH + 1, :],
                                vt[(b, st)][:, 2 * j + h, :],
                                pts[st][:, h * CW : (h + 1) * CW],
                                start=(st == 0),
                                stop=(st == KD - 1),
                            )

                pts = []
                for blk in range(KD // 2):
                    two = (2 * blk, 2 * blk + 1)
                    for st in two:
                        # one [128,1024] 2-bank tile per head-pair: both 64x128
                        # row-tiled matmuls become ready together -> execute
                        # concurrently; one exp drains both banks.
                        sc = scp.tile([P, 2 * CW], F32, tag="sc", name="sc", bufs=2)
                        for h in range(2):
                            r0 = h * DH
                            nc.tensor.matmul(
                                sc[:, h * CW : (h + 1) * CW],
                                ktj[r0 : r0 + DH, st * P : (st + 1) * P],
                                qjc[r0 : r0 + DH, :],
                                start=True,
                                stop=True,
                            )
                        pt = ptpool.tile([P, 2 * CW], BF16, tag="pt", name="pt", bufs=4)
                        nc.scalar.activation(pt[:], sc[:], EXPF, scale=0.125)
                        pts.append(pt)
                    drain(1)
                    # PVs lag one block: the j-boundary WAR stall on the pv
                    # banks (previous normalize still reading) is absorbed by
                    # this j's first pairs + fill instead of idling the PE.
                    if blk > 0:
                        pv_two((2 * blk - 2, 2 * blk - 1))
                pv_two((KD - 2, KD - 1))
                normalize(b, c, j, pva, pvb)

            # ---- head: QK proj of (b0, j0) emitted directly; V + j1 proj
            # queued so early score pairs preempt them by priority ----
            for key, u in (
                (("q", 0, 0, 0), q_unit(0, 0, 0)),
                (("k", 0, 0, 0), k_unit(0, 0, 0)),
                (("k", 0, 0, 1), k_unit(0, 0, 1)),
            ):
                u()
                done.add(key)
            push(("q", 0, 1, 0), q_unit(0, 1, 0))
            push(("k", 0, 1, 0), k_unit(0, 1, 0))
            push(("k", 0, 1, 1), k_unit(0, 1, 1))
            for mt in range(KD):
                push(("v", 0, mt, 0), v_unit(0, mt, 0))
            for mt in range(KD):
                push(("v", 0, mt, 1), v_unit(0, mt, 1))

            # ---- main loops ----
            for b in range(NB):
                for c in range(NCH):
                    for j in range(KD):
                        # schedule fill production
                        if c == 0:
                            if j < KD - 1:
                                if not (b == 0 and j == 0):  # j1 pre-queued in head
                                    push(("q", b, j + 1, 0), q_unit(b, j + 1, 0))
                                    push(("k", b, j + 1, 0), k_unit(b, j + 1, 0))
                                    push(("k", b, j + 1, 1), k_unit(b, j + 1, 1))
                            else:
                                push(("q", b, 0, 1), q_unit(b, 0, 1))
                            if j == 6 and b + 1 < NB:
                                # next batch V-proj early: feeds the thin
                                # c0-phase tail blocks
                                for mt in range(KD):
                                    push(("v", b + 1, mt, 0), v_unit(b + 1, mt, 0))
                        else:
                            if j < KD - 1:
                                push(("q", b, j + 1, 1), q_unit(b, j + 1, 1))
                            if j == 0 and b + 1 < NB:
                                for mt in range(KD):
                                    push(("v", b + 1, mt, 1), v_unit(b + 1, mt, 1))
                                    if mt % 2 == 0:
                                        push(("o", b, 0, mt // 2), out_unit(b, 0, mt // 2))
                                    else:
                                        push(("o", b, 0, mt // 2 + 4), out_unit(b, 0, mt // 2 + 4))
                            if j == 0 and b + 1 == NB:
                                for mo in range(KD):
                                    push(("o", b, 0, mo), out_unit(b, 0, mo))
                            if j == 4 and b + 1 < NB:
                                push(("q", b + 1, 0, 0), q_unit(b + 1, 0, 0))
                                push(("k", b + 1, 0, 0), k_unit(b + 1, 0, 0))
                                push(("k", b + 1, 0, 1), k_unit(b + 1, 0, 1))
                        if b == 1 and c == 0 and j == 0:
                            for mo in range(4):
                                push(("o", 0, 1, mo), out_unit(0, 1, mo))
                        if b == 1 and c == 0 and j == 5:
                            for mo in range(4, KD):
                                push(("o", 0, 1, mo), out_unit(0, 1, mo))
                        attention_cj(b, c, j)

            # ---- tail: last batch / last chunk output projection ----
            drain(len(fill))
            for mo in range(KD):
                out_unit(NB - 1, NCH - 1, mo, split_last=True)()

    nc.compile()
    return nc


def _get_nc():
    global _NC_CACHE
    if _NC_CACHE is None:
        _NC_CACHE = build_nc()
    return _NC_CACHE


def kernel(hidden_states, Wq, bq, Wk, bk, Wv, bv, Wo, bo):
    global LAST_RESULTS
    bf = ml_dtypes.bfloat16
    hs = np.asarray(hidden_states, np.float32)
    Wq = np.asarray(Wq, np.float32)
    Wk = np.asarray(Wk, np.float32)
    Wv = np.asarray(Wv, np.float32)
    Wo = np.asarray(Wo, np.float32)
    bq = np.asarray(bq, np.float32)
    bk = np.asarray(bk, np.float32)
    bv = np.asarray(bv, np.float32)
    bo = np.asarray(bo, np.float32)

    wqt = np.ascontiguousarray(Wq.T).astype(bf)
    wkt = np.ascontiguousarray(Wk.T).astype(bf)
    wvt = np.ascontiguousarray(Wv.T).astype(bf)
    wot = np.ascontiguousarray(Wo.T).astype(bf)
    bo2 = (bo + Wo @ bv).astype(np.float32)

    bpc = hs.shape[0] // NCORES  # batches per core
    in_maps = []
    for c in range(NCORES):
        xc = hs[c * bpc : (c + 1) * bpc].reshape(bpc * TB, D)
        in_maps.append(
            {
                "xt": np.ascontiguousarray(xc.T).astype(bf),
                "wqt": wqt,
                "wkt": wkt,
                "wvt": wvt,
                "wot": wot,
                "bq": bq,
                "bk": bk,
                "bo2": bo2,
            }
        )

    nc = _get_nc()
    res = run_bass_kernel_spmd(
        nc,
        in_maps,
        core_ids=list(range(NCORES)),
        trace=TRACE,
        **TRACE_KWARGS,
    )
    LAST_RESULTS = res

    out = np.empty((hs.shape[0], TB, D), np.float32)
    for c in range(NCORES):
        ot = res.results[c]["outt"]  # [D, T]
        for b in range(bpc):
            out[c * bpc + b] = ot[:, b * TB : (b + 1) * TB].T
    return out


# revision 46
# speedup vs baseline: 1.0422x; 1.0071x over previous
"""Multi-head attention forward (B=16, S=1024, d=1024, H=16, Dh=64) on 8
Trainium2 NeuronCores, data-parallel over batch (2 batches per core).

Device kernel (per core, bf16 matmuls, fp32 accumulate):
  inputs (host-prepped): XT [d, 2048] = hidden[2c:2c+2].reshape(2048,d).T,
  WqT/WkT/WvT = W.T [in, out], WoT = Wo.T [dv, o]  (all bf16),
  bq, bk [1024] f32, bo2 = bo + Wo @ bv  (bv folded: softmax rows sum to 1).

Key structure (vs earlier version): query chunks (c) are OUTER, head pairs
(j) inner.  Scores for a head pair go into ONE [128,1024] PSUM tile
(h0 -> bank A cols 0:512, h1 -> bank B) so the two 64x128 row-tiled matmuls
become ready together and execute CONCURRENTLY on the PE (row tiles 0/64),
and a single [128,1024] exp drains both.  pvps needs only 2 banks per (c,j)
so PSUM = scores ring 4 + pv 2 + proj 2 = 8 banks.  Projections / V-proj /
out-proj are emitted as 8-matmul fill units drained between score blocks.
"""

import numpy as np
import ml_dtypes

import concourse.bass as bass
import concourse.mybir as mybir
import concourse.tile as tile
from concourse import bacc
from concourse.bass_utils import run_bass_kernel_spmd

P = 128
D = 1024
T = 2048  # tokens per core
TB = 1024  # tokens per batch (= S)
H = 16
DH = 64
KD = D // P  # 8 partition-tiles of the d/dv/s dims
NB = T // TB  # batches per core
CW = 512  # query-chunk width (one PSUM bank of fp32)
NCH = TB // CW  # 2 query chunks per batch
NCORES = 8

BF16 = mybir.dt.bfloat16
F32 = mybir.dt.float32
EXPF = mybir.ActivationFunctionType.Exp
IDF = mybir.ActivationFunctionType.Identity
MULT = mybir.AluOpType.mult

# test.py hooks
TRACE = False
TRACE_KWARGS = {}
LAST_RESULTS = None

_NC_CACHE = None


def build_nc():
    from collections import deque
    from contextlib import ExitStack

    nc = bacc.Bacc("TRN2", target_bir_lowering=False, debug=False, num_devices=NCORES)

    xt_d = nc.dram_tensor("xt", [D, T], BF16, kind="ExternalInput")
    wqt_d = nc.dram_tensor("wqt", [D, D], BF16, kind="ExternalInput")
    wkt_d = nc.dram_tensor("wkt", [D, D], BF16, kind="ExternalInput")
    wvt_d = nc.dram_tensor("wvt", [D, D], BF16, kind="ExternalInput")
    wot_d = nc.dram_tensor("wot", [D, D], BF16, kind="ExternalInput")
    bq_d = nc.dram_tensor("bq", [D], F32, kind="ExternalInput")
    bk_d = nc.dram_tensor("bk", [D], F32, kind="ExternalInput")
    bo2_d = nc.dram_tensor("bo2", [D], F32, kind="ExternalInput")
    outt_d = nc.dram_tensor("outt", [D, T], F32, kind="ExternalOutput")

    with tile.TileContext(nc) as tc:
        with ExitStack() as ctx:
            # PSUM first so the 2-bank scores tiles land bank-aligned.
            scp = ctx.enter_context(tc.tile_pool(name="sc", bufs=2, space="PSUM"))
            pvp = ctx.enter_context(tc.tile_pool(name="pv", bufs=1, space="PSUM"))
            prp = ctx.enter_context(tc.tile_pool(name="pr", bufs=2, space="PSUM"))
            wpool = ctx.enter_context(tc.tile_pool(name="w", bufs=1))
            xpool = ctx.enter_context(tc.tile_pool(name="x", bufs=1))
            qkpool = ctx.enter_context(tc.tile_pool(name="qk", bufs=1))
            vpool = ctx.enter_context(tc.tile_pool(name="v", bufs=2))
            ptpool = ctx.enter_context(tc.tile_pool(name="pt", bufs=4))
            cpool = ctx.enter_context(tc.tile_pool(name="ctx", bufs=2))
            npool = ctx.enter_context(tc.tile_pool(name="norm", bufs=2))
            opool = ctx.enter_context(tc.tile_pool(name="out", bufs=2))
            spool = ctx.enter_context(tc.tile_pool(name="small", bufs=1))

            # ---- global tiles + DMA loads (interleaved for fast rampup) ----
            xt = [xpool.tile([P, T], BF16, tag=f"xt{k}", name=f"xt{k}") for k in range(KD)]
            wq, wk, wv, wo = (
                [wpool.tile([P, D], BF16, tag=f"w{nm}{k}", name=f"w{nm}{k}") for k in range(KD)]
                for nm in "qkvo"
            )
            # batch-0 halves of xt + whole wq/wk first (unblocks first QK
            # proj), then wv (V-proj), then batch-1 xt halves, wo last.
            for k in range(KD):
                nc.sync.dma_start(xt[k][:, 0:TB], xt_d[k * P : (k + 1) * P, 0:TB])
                nc.sync.dma_start(wq[k][:], wqt_d[k * P : (k + 1) * P, :])
                nc.sync.dma_start(wk[k][:], wkt_d[k * P : (k + 1) * P, :])
            bq_sb = spool.tile([P, KD], F32, tag="bq", name="bq_sb")
            bk_sb = spool.tile([P, KD], F32, tag="bk", name="bk_sb")
            bo_sb = spool.tile([P, KD], F32, tag="bo", name="bo_sb")
            for sb, dr in ((bq_sb, bq_d), (bk_sb, bk_d)):
                nc.sync.dma_start(sb[:], dr.rearrange("(o p) -> p o", p=P))
            for k in range(KD):
                nc.sync.dma_start(wv[k][:], wvt_d[k * P : (k + 1) * P, :])
            nc.sync.dma_start(bo_sb[:], bo2_d.rearrange("(o p) -> p o", p=P))
            for k in range(KD):
                nc.sync.dma_start(xt[k][:, TB:T], xt_d[k * P : (k + 1) * P, TB:T])
            for k in range(KD):
                nc.sync.dma_start(wo[k][:], wot_d[k * P : (k + 1) * P, :])
            # h1-half of wo's last tile, staged at partitions 0:64 so the tail
            # can consume the final head's ctx without the partition-shift DMA
            wo7b = wpool.tile([DH, D], BF16, tag="wo7b", name="wo7b")
            nc.sync.dma_start(wo7b[:], wot_d[D - DH : D, :])

            # ---- persistent per-batch state ----
            vt = {}  # (b, mt) -> v tile [P, H, DH+1]
            kt = {}  # (b, j) -> K^T tile [P, TB]
            qt = {}  # (b, j, c) -> Q^T chunk tile [P, CW]
            ctxts = {}  # (b, j) -> ctx^T tile [P, TB]

            fill = deque()  # (key, closure)
            done = set()

            def push(key, closure):
                fill.append((key, closure))

            def drain(n):
                for _ in range(min(n, len(fill))):
                    k, f = fill.popleft()
                    f()
                    done.add(k)

            def need(key):
                # force-drain (in FIFO order) until `key` has been emitted;
                # guarantees emission-order dependencies for dict tiles.
                while key not in done:
                    assert fill, f"need({key}) but fill queue empty"
                    k, f = fill.popleft()
                    f()
                    done.add(k)

            # ---- fill units (each ~8 matmuls + epilogue) ----
            def v_unit(b, mt, ch):
                def emit():
                    if (b, mt) not in vt:
                        vt[(b, mt)] = vpool.tile(
                            [P, H, DH + 1], BF16, tag=f"v{mt}", name=f"v{mt}", bufs=2
                        )
                        nc.vector.memset(vt[(b, mt)][:, :, DH : DH + 1], 1.0)
                    ps = prp.tile([P, CW], F32, tag="pr", name="prv")
                    for k in range(KD):
                        nc.tensor.matmul(
                            ps[:],
                            xt[k][:, (b * KD + mt) * P : (b * KD + mt + 1) * P],
                            wv[k][:, ch * CW : (ch + 1) * CW],
                            start=(k == 0),
                            stop=(k == KD - 1),
                        )
                    nc.vector.tensor_copy(
                        vt[(b, mt)][:, ch * 8 : (ch + 1) * 8, 0:DH],
                        ps.rearrange("p (h d) -> p h d", d=DH),
                    )

                return emit

            def k_unit(b, j, ch):
                def emit():
                    if (b, j) not in kt:
                        kt[(b, j)] = qkpool.tile(
                            [P, TB], BF16, tag=f"k{j}", name=f"kt{j}", bufs=1
                        )
                    ps = prp.tile([P, CW], F32, tag="pr", name="prk")
                    for k in range(KD):
                        nc.tensor.matmul(
                            ps[:],
                            wk[k][:, j * P : (j + 1) * P],
                            xt[k][:, b * TB + ch * CW : b * TB + (ch + 1) * CW],
                            start=(k == 0),
                            stop=(k == KD - 1),
                        )
                    nc.scalar.activation(
                        kt[(b, j)][:, ch * CW : (ch + 1) * CW],
                        ps[:],
                        IDF,
                        bias=bk_sb[:, j : j + 1],
                    )

                return emit

            def q_unit(b, j, c):
                def emit():
                    qt[(b, j, c)] = qkpool.tile(
                        [P, CW], BF16, tag=f"q{j}", name=f"qt{j}", bufs=1
                    )
                    ps = prp.tile([P, CW], F32, tag="pr", name="prq")
                    for k in range(KD):
                        nc.tensor.matmul(
                            ps[:],
                            wq[k][:, j * P : (j + 1) * P],
                            xt[k][:, b * TB + c * CW : b * TB + (c + 1) * CW],
                            start=(k == 0),
                            stop=(k == KD - 1),
                        )
                    nc.scalar.activation(
                        qt[(b, j, c)][:], ps[:], IDF, bias=bq_sb[:, j : j + 1]
                    )

                return emit

            def out_unit(b, c, mo, split_last=False):
                def emit():
                    ps = prp.tile([P, CW], F32, tag="pr", name="pro")
                    last = KD - 1
                    for k in range(last):
                        nc.tensor.matmul(
                            ps[:],
                            wo[k][:, mo * P : (mo + 1) * P],
                            ctxts[(b, k)][:, c * CW : (c + 1) * CW],
                            start=(k == 0),
                            stop=False,
                        )
                    if split_last:
                        # final tile in two 64-deep halves, both at row-tile
                        # position 0 (same row group -> serialized, no PSUM
                        # bank race); h1 comes from the direct staging tile
                        nc.tensor.matmul(
                            ps[:],
                            wo[last][0:DH, mo * P : (mo + 1) * P],
                            ctxts[(b, last)][0:DH, c * CW : (c + 1) * CW],
                            start=False,
                            stop=False,
                        )
                        nc.tensor.matmul(
                            ps[:],
                            wo7b[:, mo * P : (mo + 1) * P],
                            ctx_h1_last[0],
                            start=False,
                            stop=True,
                        )
                    else:
                        nc.tensor.matmul(
                            ps[:],
                            wo[last][:, mo * P : (mo + 1) * P],
                            ctxts[(b, last)][:, c * CW : (c + 1) * CW],
                            start=False,
                            stop=True,
                        )
                    osb = opool.tile([P, CW], F32, tag="osb", name="osb")
                    nc.scalar.activation(osb[:], ps[:], IDF, bias=bo_sb[:, mo : mo + 1])
                    nc.sync.dma_start(
                        outt_d[
                            mo * P : (mo + 1) * P,
                            b * TB + c * CW : b * TB + (c + 1) * CW,
                        ],
                        osb[:],
                    )

                return emit

            # ---- attention inner loop ----
            ctx_h1_last = [None]

            def normalize(b, c, j, pva, pvb):
                if (b, j) not in ctxts:
                    ctxts[(b, j)] = cpool.tile(
                        [P, TB], BF16, tag=f"ctxt{j}", name=f"ctxt{j}", bufs=2
                    )
                ctile = ctxts[(b, j)]
                final = b == NB - 1 and c == NCH - 1 and j == KD - 1
                for h, pv_t in ((0, pva), (1, pvb)):
                    rs = npool.tile([1, CW], F32, tag="rs", name="rs", bufs=1)
                    nc.vector.tensor_copy(rs[:], pv_t[DH : DH + 1, :])
                    rr = npool.tile([1, CW], F32, tag="rr", name="rr", bufs=1)
                    nc.vector.reciprocal_approx_fast(rr[:], rs[:])
                    rb = npool.tile([DH, CW], F32, tag="rb", name="rb", bufs=1)
                    nc.gpsimd.partition_broadcast(rb[:], rr[:])
                    if h == 0:
                        nc.vector.tensor_tensor(
                            ctile[0:DH, c * CW : (c + 1) * CW],
                            pv_t[0:DH, :],
                            rb[:],
                            MULT,
                        )
                    elif final:
                        # keep the very last head's ctx at partitions 0:64 so
                        # the tail consumes it without the shift DMA
                        chl = npool.tile([DH, CW], BF16, tag="chl", name="chl", bufs=1)
                        nc.vector.tensor_tensor(chl[:], pv_t[0:DH, :], rb[:], MULT)
                        ctx_h1_last[0] = chl
                    else:
                        ch = npool.tile([DH, CW], BF16, tag="ch", name="ch", bufs=1)
                        nc.vector.tensor_tensor(ch[:], pv_t[0:DH, :], rb[:], MULT)
                        nc.sync.dma_start(
                            ctile[DH:P, c * CW : (c + 1) * CW], ch[:]
                        )

            def attention_cj(b, c, j):
                need(("q", b, j, c))
                need(("k", b, j, 0))
                need(("k", b, j, 1))
                pva = pvp.tile([P, CW], F32, tag="pva", name="pva", bufs=1)
                pvb = pvp.tile([P, CW], F32, tag="pvb", name="pvb", bufs=1)
                ktj = kt[(b, j)]
                qjc = qt[(b, j, c)]
                def pv_two(two):
                    for st in two:
                        need(("v", b, st, j // 4))
                        for h, pv_t in ((0, pva), (1, pvb)):
                            nc.tensor.matmul(
                                pv_t[0 : DH + 1, :],
                                vt[(b, st)][:, 2 * j + h, :],
                                pts[st][:, h * CW : (h + 1) * CW],
                                start=(st == 0),
                                stop=(st == KD - 1),
                            )

                pts = []
                for blk in range(KD // 2):
                    two = (2 * blk, 2 * blk + 1)
                    for st in two:
                        # one [128,1024] 2-bank tile per head-pair: both 64x128
                        # row-tiled matmuls become ready together -> execute
                        # concurrently; one exp drains both banks.
                        sc = scp.tile([P, 2 * CW], F32, tag="sc", name="sc", bufs=2)
                        for h in range(2):
                            r0 = h * DH
                            nc.tensor.matmul(
                                sc[:, h * CW : (h + 1) * CW],
                                ktj[r0 : r0 + DH, st * P : (st + 1) * P],
                                qjc[r0 : r0 + DH, :],
                                start=True,
                                stop=True,
                            )
                        pt = ptpool.tile([P, 2 * CW], BF16, tag="pt", name="pt", bufs=4)
                        nc.scalar.activation(pt[:], sc[:], EXPF, scale=0.125)
                        pts.append(pt)
                    drain(1)
                    # PVs lag one block: the j-boundary WAR stall on the pv
                    # banks (previous normalize still reading) is absorbed by
                    # this j's first pairs + fill instead of idling the PE.
                    if blk > 0:
                        pv_two((2 * blk - 2, 2 * blk - 1))
                pv_two((KD - 2, KD - 1))
                normalize(b, c, j, pva, pvb)

            # ---- head: QK proj of (b0, j0) emitted directly; V + j1 proj
            # queued so early score pairs preempt them by priority ----
            for key, u in (
                (("q", 0, 0, 0), q_unit(0, 0, 0)),
                (("k", 0, 0, 0), k_unit(0, 0, 0)),
                (("k", 0, 0, 1), k_unit(0, 0, 1)),
            ):
                u()
                done.add(key)
            push(("q", 0, 1, 0), q_unit(0, 1, 0))
            push(("k", 0, 1, 0), k_unit(0, 1, 0))
            push(("k", 0, 1, 1), k_unit(0, 1, 1))
            for mt in range(KD):
                push(("v", 0, mt, 0), v_unit(0, mt, 0))
            for mt in range(KD):
                push(("v", 0, mt, 1), v_unit(0, mt, 1))

            # ---- main loops ----
            for b in range(NB):
                for c in range(NCH):
                    for j in range(KD):
                        # schedule fill production
                        if c == 0:
                            if j < KD - 1:
                                if not (b == 0 and j == 0):  # j1 pre-queued in head
                                    push(("q", b, j + 1, 0), q_unit(b, j + 1, 0))
                                    push(("k", b, j + 1, 0), k_unit(b, j + 1, 0))
                                    push(("k", b, j + 1, 1), k_unit(b, j + 1, 1))
                            else:
                                push(("q", b, 0, 1), q_unit(b, 0, 1))
                            if j == 6 and b + 1 < NB:
                                # next batch V-proj early: feeds the thin
                                # c0-phase tail blocks
                                for mt in range(KD):
                                    push(("v", b + 1, mt, 0), v_unit(b + 1, mt, 0))
                        else:
                            if j < KD - 1:
                                push(("q", b, j + 1, 1), q_unit(b, j + 1, 1))
                            if j == 0 and b + 1 < NB:
                                for mt in range(KD):
                                    push(("v", b + 1, mt, 1), v_unit(b + 1, mt, 1))
                                    if mt % 2 == 0:
                                        push(("o", b, 0, mt // 2), out_unit(b, 0, mt // 2))
                                    else:
                                        push(("o", b, 0, mt // 2 + 4), out_unit(b, 0, mt // 2 + 4))
                            if j == 0 and b + 1 == NB:
                                for mo in range(KD):
                                    push(("o", b, 0, mo), out_unit(b, 0, mo))
                            if j == 4 and b + 1 < NB:
                                push(("q", b + 1, 0, 0), q_unit(b + 1, 0, 0))
                                push(("k", b + 1, 0, 0), k_unit(b + 1, 0, 0))
                                push(("k", b + 1, 0, 1), k_unit(b + 1, 0, 1))
                        if b == 1 and c == 0 and j == 0:
                            for mo in range(4):
                                push(("o", 0, 1, mo), out_unit(0, 1, mo))
                        if b == 1 and c == 0 and j == 5:
                            for mo in range(4, KD):
                                push(("o", 0, 1, mo), out_unit(0, 1, mo))
                        attention_cj(b, c, j)

            # ---- tail: last batch / last chunk output projection ----
            drain(len(fill))
            for mo in range(KD):
                out_unit(NB - 1, NCH - 1, mo, split_last=True)()

    nc.compile()
    return nc


def _get_nc():
    global _NC_CACHE
    if _NC_CACHE is None:
        _NC_CACHE = build_nc()
    return _NC_CACHE


def kernel(hidden_states, Wq, bq, Wk, bk, Wv, bv, Wo, bo):
    global LAST_RESULTS
    bf = ml_dtypes.bfloat16
    hs = np.asarray(hidden_states, np.float32)
    Wq = np.asarray(Wq, np.float32)
    Wk = np.asarray(Wk, np.float32)
    Wv = np.asarray(Wv, np.float32)
    Wo = np.asarray(Wo, np.float32)
    bq = np.asarray(bq, np.float32)
    bk = np.asarray(bk, np.float32)
    bv = np.asarray(bv, np.float32)
    bo = np.asarray(bo, np.float32)

    wqt = np.ascontiguousarray(Wq.T).astype(bf)
    wkt = np.ascontiguousarray(Wk.T).astype(bf)
    wvt = np.ascontiguousarray(Wv.T).astype(bf)
    wot = np.ascontiguousarray(Wo.T).astype(bf)
    bo2 = (bo + Wo @ bv).astype(np.float32)

    bpc = hs.shape[0] // NCORES  # batches per core
    in_maps = []
    for c in range(NCORES):
        xc = hs[c * bpc : (c + 1) * bpc].reshape(bpc * TB, D)
        in_maps.append(
            {
                "xt": np.ascontiguousarray(xc.T).astype(bf),
                "wqt": wqt,
                "wkt": wkt,
                "wvt": wvt,
                "wot": wot,
                "bq": bq,
                "bk": bk,
                "bo2": bo2,
            }
        )

    nc = _get_nc()
    res = run_bass_kernel_spmd(
        nc,
        in_maps,
        core_ids=list(range(NCORES)),
        trace=TRACE,
        **TRACE_KWARGS,
    )
    LAST_RESULTS = res

    out = np.empty((hs.shape[0], TB, D), np.float32)
    for c in range(NCORES):
        ot = res.results[c]["outt"]  # [D, T]
        for b in range(bpc):
            out[c * bpc + b] = ot[:, b * TB : (b + 1) * TB].T
    return out
